# revision 43
# baseline (speedup 1.0000x reference)
"""Trainium2 Bass kernel for a NeuralODE (forward-Euler scan over a tiny MLP).

Reference computation (per batch row x of `initial`):
    h0 = x @ Wi + bi                                  # [32]
    h_{t+1} = h_t + dt_t * f(h_t),  t = 0..T-2
    f(h) = tanh(tanh(tanh(h@W0+b0)@W1+b1)@W2+b2) @ W3 + b3
    out[t] = h_t @ Wl + bl                            # [8], t = 0..T-1

Device reformulation (exact in exact arithmetic): track the projected state
    p_t = W0^T h_t   (15-dim)     o_t = Wl^T h_t + bl   (8-dim = the output!)
since h_t only ever enters through W0 (layer 0) and Wl (readout):
    z  = tanh(p + b0); z = tanh(z@W1+b1); z2 = tanh(z@W2+b2)
    p += dt * (z2 @ (W3@W0) + b3@W0)
    o += dt * (z2 @ (W3@Wl) + b3@Wl)
This removes the h->z matmul and the separate trajectory projection pass:
the o-part of the state IS the output trajectory.

The scan is latency-bound: each step is a 6-hop cross-engine cycle
(act->mm->act->mm->act->mmG) whose per-hop cost is dominated by fixed
constants (act SBUF access 2x185ns, PE pipeline 173ns, sem hops ~50ns), so
the layout minimizes the per-instruction column width on the critical acts:

Accum layout (constant dt, the harness case): stride-16, 8 chunks.
  512 batch rows per core = 8 chunks x 64 batch columns.
  p-state tile [128 part, 64 cols]: chunk c occupies partitions 16c..16c+14;
  row 127 is the constant-1 bias row of z2 (chunk 7's spare). The o-state
  lives separately in po [64, 8c+j rows, cols], accumulated in its own PSUM
  bank per stream (PSUM start=True resets at bank granularity, so sharing a
  bank across streams would wipe earlier streams' o0 seeds).
  Weights are 128x128 / 128x64 block-diagonal matrices (host-assembled):
  W1bd/W2bd blocks [15,15] at (16c,16c); Gbd blocks [15,15] = dt*(W3@W0) at
  (16c,16c) plus row 127 = dt*(b3@W0) per chunk; Gbdo [16c+i, 8c+j] =
  dt*(W3@Wl)[i,j] plus row 127 = dt*(b3@Wl).
  Step: act0 -> mm1 -> act1 -> mm2 -> act2 -> mmG (accumulates pg in PSUM,
  start=False) + mmGo (accumulates po off-chain); a DVE copy snapshots
  o_{t-1} into a [64, TBUF*w] SBUF ring which DMA-drains to DRAM scratch
  [64, T*64]; the host transposes scratch (c,o;t,n) -> out[c*64+n, t, o].

  All per-core constants ship as ONE dram tensor / ONE dma_start (each
  dma_start costs ~650ns of serialized SP-sequencer issue time; the former
  11-DMA preamble burned ~7us before the first matmul).

Non-accum fallback (variable dt; unused by the harness): stride-32,
4 chunks of 128 cols, full 23-row state in SBUF ring blocks, DVE
scalar_tensor_tensor update with per-step dt from SBUF.

Post-compile, _fuse_act_waits attaches each act's spilled cross-engine wait
directly to the instruction (TRN2 allows 1 wait per instruction; the
scheduler spills the second onto a standalone EventSemaphore, which
serializes sem-fire -> SEQ-decode -> engine on the critical chain).
"""

from contextlib import ExitStack

import numpy as np

B, T = 4096, 1000
INIT_DIM, HID, HH, OUT = 16, 32, 15, 8
NCORES = 8
BSH = B // NCORES          # 512 batch rows per core
NSTREAM = 2                # independent dependency chains per core
TBUF = 4                   # time slots per ring block (4 divides 1000)
FUSE_WAITS = True          # post-compile: attach spilled act waits

# ---- accum (stride-16) layout
NCHA = 8                   # chunks per core
CA = 64                    # batch cols per core (8 x 64 = 512)
ONES_A = 127               # z2 constant-one row (chunk 7 spare)
ACT_HI_A = 127             # act2 writes partitions [0, ACT_HI_A)
# column offsets inside the consolidated per-core constant tensor; the
# "hot" prefix [0, CO_SPLIT) covers everything the seed matmuls and act0
# need and ships as its own DMA so the chain starts before the rest lands
CO_ID, CO_S0, CO_S0O, CO_BZ = 0, 128, 192, 256
CO_SPLIT = 260
CO_W1, CO_W2, CO_G, CO_GO, CO_Z2 = 260, 388, 516, 644, 708
CST_COLS = 772

# ---- non-accum (stride-32) layout
NCH = 4
ONES_ROW = 124
ACT_HI = 111


def _fuse_act_waits(nc):
    """Merge each Act-engine standalone EventSemaphore into the following
    Activation instruction.

    Every per-step act carries (a) a same-engine ordering wait
    S[Activation]>=k (WAW on its z output vs the previous step's act,
    guaranteed anyway by in-order engine execution and FIFO SBUF writes)
    and (b) a spilled cross-engine data wait. Attaching (b) directly to
    the act lets the sequencer pre-decode and queue it at the engine, so
    the sem fires straight into execution (~70-90ns per critical hop).
    """
    from concourse import mybir

    fn = nc.m.functions[0]
    for blk in fn.blocks:
        insts = blk.instructions
        drop = []
        pending = None          # (index, EventSemaphore) awaiting its act
        for idx, ins in enumerate(insts):
            if not str(ins.engine).endswith("Activation"):
                continue
            op = ins.opcode
            if op == "EventSemaphore":
                si = ins.sync_info
                if (pending is None and si is not None
                        and len(si.on_wait) == 1 and not si.on_update
                        and not si.on_wait[0].ant_name.startswith("Activation")):
                    pending = (idx, ins)
                else:
                    pending = None
            elif op == "Activation" and pending is not None:
                asi = ins.sync_info
                psi = pending[1].sync_info
                if (asi is not None and len(asi.on_wait) == 1
                        and asi.on_wait[0].ant_name.startswith("Activation")):
                    ins.sync_info = mybir.SyncInfo(
                        on_wait=[psi.on_wait[0]],
                        on_update=list(asi.on_update))
                    drop.append(pending[0])
                pending = None
            else:
                pending = None
        for idx in reversed(drop):
            del insts[idx]


def build_program(t_total=T, tbuf=TBUF, nstream=NSTREAM, accum=True,
                  repeats=1):
    """Build + compile the per-core Bass program (SPMD: same on all cores).

    `nstream` independent dependency chains, each covering a disjoint
    column-slice of the batch, interleave on the engines to hide the
    per-step cross-engine latency (the serial chain is ~2us/step).
    """
    import concourse.tile as tile
    from concourse import bacc, mybir

    F32 = mybir.dt.float32
    Tanh = mybir.ActivationFunctionType.Tanh

    nc = bacc.Bacc("TRN2", target_bir_lowering=False, debug=False)

    ncols = CA if accum else 128
    scr_rows = NCHA * OUT if accum else NCH * OUT
    nb = t_total // tbuf
    assert nb * tbuf == t_total
    base = ncols // nstream
    ws = [base + (1 if s < ncols % nstream else 0) for s in range(nstream)]

    scr = nc.dram_tensor("oscr", [scr_rows, t_total * ncols], F32,
                         kind="ExternalOutput")
    if accum:
        cst = nc.dram_tensor("cst", [128, CST_COLS], F32,
                             kind="ExternalInput")
    else:
        s0 = nc.dram_tensor("s0", [128, 128], F32, kind="ExternalInput")
        w1 = nc.dram_tensor("w1bd", [128, 128], F32, kind="ExternalInput")
        w2 = nc.dram_tensor("w2bd", [128, 128], F32, kind="ExternalInput")
        gm = nc.dram_tensor("gbd", [128, 128], F32, kind="ExternalInput")
        bzt = nc.dram_tensor("bz", [128, 4], F32, kind="ExternalInput")
        z2i = nc.dram_tensor("z2init", [128, 128], F32, kind="ExternalInput")
        dts = nc.dram_tensor("dts", [128, t_total - 1], F32,
                             kind="ExternalInput")

    with tile.TileContext(nc) as tc, ExitStack() as ctx:
        const = ctx.enter_context(tc.tile_pool(name="const", bufs=1))
        rings = [ctx.enter_context(tc.tile_pool(name=f"ring{s}", bufs=2))
                 for s in range(nstream)]
        psum = ctx.enter_context(tc.tile_pool(name="psum", bufs=1, space="PSUM"))

        if accum:
            cst_sb = const.tile([128, CST_COLS], F32, tag="cst")
            # hot prefix (seed + act0 inputs) on the SP queue, the rest on
            # the Activation-sequencer queue: the two issues overlap, and the
            # seed matmuls only wait for the small hot transfer
            nc.sync.dma_start(cst_sb[:, 0:CO_SPLIT],
                              cst.ap()[:, 0:CO_SPLIT])
            nc.scalar.dma_start(cst_sb[:, CO_SPLIT:CST_COLS],
                                cst.ap()[:, CO_SPLIT:CST_COLS])
            _c = cst_sb[:]
            w1_sb = _c[:, CO_W1:CO_W1 + 128]
            w2_sb = _c[:, CO_W2:CO_W2 + 128]
            g_sb = _c[:, CO_G:CO_G + 128]
            bz_sb = _c[:, CO_BZ:CO_BZ + 4]
            id_sb = _c[:, CO_ID:CO_ID + 128]
            s0_sb = _c[:, CO_S0:CO_S0 + CA]
            go_sb = _c[:, CO_GO:CO_GO + CA]
            s0o_sb = _c[:, CO_S0O:CO_S0O + CA]
            act_hi = ACT_HI_A
        else:
            w1t = const.tile([128, 128], F32, tag="w1")
            w2t = const.tile([128, 128], F32, tag="w2")
            gt = const.tile([128, 128], F32, tag="g")
            bzs = const.tile([128, 4], F32, tag="bz")
            dts_sb = const.tile([128, t_total - 1], F32, tag="dts")
            nc.sync.dma_start(w1t[:], w1.ap())
            nc.sync.dma_start(w2t[:], w2.ap())
            nc.sync.dma_start(gt[:], gm.ap())
            nc.sync.dma_start(bzs[:], bzt.ap())
            nc.sync.dma_start(dts_sb[:], dts.ap())
            w1_sb, w2_sb, g_sb, bz_sb = w1t[:], w2t[:], gt[:], bzs[:]
            act_hi = ACT_HI

        class Stream:
            pass

        # nstream=2 fits 4 private PSUM banks per stream. For nstream>=3,
        # merge p1/p2 into one transient bank per stream (mm2's start=True
        # reset of the bank is chain-ordered behind act1's read of p1) and
        # share a single po bank across streams, seeded by ONE start=True
        # matmul over all columns (per-stream seeds would wipe each other:
        # PSUM start=True resets has_written at bank granularity).
        share_psum = nstream >= 3 and accum
        po_all = None
        if share_psum:
            po_all = psum.tile([NCHA * OUT, ncols], F32, tag="po",
                               name="po_all")
            nc.tensor.matmul(po_all[:], id_sb[0:NCHA * OUT, 0:NCHA * OUT],
                             s0o_sb[0:NCHA * OUT, 0:ncols],
                             start=True, stop=False, skip_group_check=True)

        streams = []
        lo = 0
        for s in range(nstream):
            w = ws[s]
            st = Stream()
            st.lo, st.w = lo, w
            lo += w
            st.z0 = const.tile([128, w], F32, tag=f"z0_{s}")
            st.z1 = const.tile([128, w], F32, tag=f"z1_{s}")
            st.p1 = psum.tile([128, w], F32, tag=f"p1_{s}")
            st.p2 = st.p1 if share_psum else psum.tile(
                [128, w], F32, tag=f"p2_{s}", name=f"p2_{s}")
            st.pg = psum.tile([128, w], F32, tag=f"pg_{s}")
            st.prev = None
            st.blk = None
            if accum:
                # z2 lives inside the consolidated const tile: its ones-row
                # (and junk-row zeros) arrive with the single cst DMA; act2
                # overwrites rows [0, ACT_HI_A) each step.
                st.z2 = _c[:, CO_Z2 + st.lo:CO_Z2 + st.lo + w]
                # p-state accumulator in PSUM (the critical chain reads only
                # this bank), seeded via identity matmul so the PSUM
                # has_written bits are set by the PE itself
                nc.tensor.matmul(st.pg[:], id_sb,
                                 s0_sb[:, st.lo:st.lo + w],
                                 start=True, stop=False, skip_group_check=True)
                # o-state accumulator outside the chain's banks: never read
                # by the chain, so snapshot copies cannot stall the next step
                if share_psum:
                    st.po = po_all[:, st.lo:st.lo + w]
                else:
                    po = psum.tile([NCHA * OUT, w], F32, tag=f"po_{s}",
                                   name=f"po_{s}")
                    st.po = po[:]
                    nc.tensor.matmul(
                        st.po, id_sb[0:NCHA * OUT, 0:NCHA * OUT],
                        s0o_sb[0:NCHA * OUT, st.lo:st.lo + w],
                        start=True, stop=False, skip_group_check=True)
            else:
                z2t = const.tile([128, w], F32, tag=f"z2_{s}",
                                 name=f"z2_{s}")
                st.z2 = z2t[:]
                nc.sync.dma_start(st.z2, z2i.ap()[:, st.lo:st.lo + w])
            streams.append(st)

        def step_accum(st, slot, blks):
            """Emit the chain producing state s_{slot}; also emit the
            snapshot copy of s_{slot-1} mid-emission so program order puts
            the next chain's act0 (a co-reader of the accumulator bank)
            ahead of the copy."""
            k1, i1 = divmod(slot - 1, tbuf)
            w = st.w
            prev_cur = st.blks[k1][:, i1 * w:(i1 + 1) * w]
            # slot 1 reads q0 straight from the SBUF constant block, so the
            # chain starts as soon as the hot DMA lands; the PSUM seed
            # matmuls only have to finish before mmG(1), five hops later
            q_in = s0_sb[:, st.lo:st.lo + w] if slot == 1 else st.pg[:]
            nc.scalar.activation(st.z0[:], q_in, Tanh, bias=bz_sb[:, 0:1])
            nc.tensor.matmul(st.p1[:], w1_sb, st.z0[:],
                             start=True, stop=True)
            # snapshot o_{slot-1} into the output ring (off the critical path)
            nc.vector.tensor_copy(prev_cur, st.po)
            nc.scalar.activation(st.z1[:], st.p1[:], Tanh, bias=bz_sb[:, 1:2])
            nc.tensor.matmul(st.p2[:], w2_sb, st.z1[:],
                             start=True, stop=True)
            nc.scalar.activation(
                st.z2[0:act_hi, :], st.p2[0:act_hi, :], Tanh,
                bias=bz_sb[0:act_hi, 2:3],
            )
            # p += (dt*G_p)^T z2 and o += (dt*G_o)^T z2, accumulated by the PE
            # (the final slot's q-update feeds nothing: q_{T} is never read,
            # so skip its mmG and let the last mmGo fire sooner)
            if slot < t_total - 1:
                nc.tensor.matmul(st.pg[:], g_sb, st.z2,
                                 start=False, stop=False,
                                 skip_group_check=True)
            nc.tensor.matmul(st.po, go_sb, st.z2,
                             start=False, stop=False, skip_group_check=True)

        def step(st, slot, k, i):
            w = st.w
            cur = st.blk[:, i * w:(i + 1) * w]
            if slot == 0:
                nc.sync.dma_start(cur, s0.ap()[:, st.lo:st.lo + w])
                st.prev = cur
                return
            nc.scalar.activation(st.z0[:], st.prev, Tanh, bias=bz_sb[:, 0:1])
            nc.tensor.matmul(st.p1[:], w1_sb, st.z0[:], start=True, stop=True)
            nc.scalar.activation(st.z1[:], st.p1[:], Tanh, bias=bz_sb[:, 1:2])
            nc.tensor.matmul(st.p2[:], w2_sb, st.z1[:], start=True, stop=True)
            nc.scalar.activation(
                st.z2[0:act_hi, :], st.p2[0:act_hi, :], Tanh,
                bias=bz_sb[0:act_hi, 2:3],
            )
            nc.tensor.matmul(st.pg[:], g_sb, st.z2, start=True, stop=True)
            nc.vector.scalar_tensor_tensor(
                cur, st.pg[:], dts_sb[:, slot - 1:slot], st.prev,
                mybir.AluOpType.mult, mybir.AluOpType.add,
            )
            st.prev = cur

        def drain(st, blk, k):
            # block k's o-rows -> DRAM scratch (non-accum, 32-stride state)
            for c in range(NCH):
                nc.sync.dma_start(
                    scr.ap().rearrange("p (t n) -> p t n", n=ncols)[
                        c * 8:(c + 1) * 8, k * tbuf:(k + 1) * tbuf,
                        st.lo:st.lo + st.w],
                    blk[32 * c + 15:32 * c + 23, :].rearrange(
                        "p (t n) -> p t n", n=st.w),
                )

        def drain_o(st, blk, k, i0=0, i1=None):
            # accum mode: blk is already [64=(c,o), tbuf*w], matching scr rows
            i1 = tbuf if i1 is None else i1
            nc.sync.dma_start(
                scr.ap().rearrange("p (t n) -> p t n", n=ncols)[
                    :, k * tbuf + i0:k * tbuf + i1, st.lo:st.lo + st.w],
                blk[:, i0 * st.w:i1 * st.w].rearrange(
                    "p (t n) -> p t n", n=st.w),
            )

        if accum:
            for s, st in enumerate(streams):
                st.blks = {}

            def get_blk(st, s_idx, k):
                if k not in st.blks:
                    st.blks[k] = rings[s_idx].tile(
                        [NCHA * OUT, tbuf * st.w], F32, tag=f"blk{s_idx}",
                        name=f"blk{s_idx}_{k}")
                return st.blks[k]

            for rep in range(repeats):
                if rep:
                    for st in streams:   # fresh ring tiles each repeat
                        st.blks = {}
                for slot in range(1, t_total):
                    k = slot // tbuf
                    for s_idx, st in enumerate(streams):
                        get_blk(st, s_idx, (slot - 1) // tbuf)
                        get_blk(st, s_idx, k)
                        step_accum(st, slot, st.blks)
                        if slot % tbuf == 0:
                            drain_o(st, st.blks[k - 1], k - 1)
            kl, il = divmod(t_total - 1, tbuf)
            # completed slots of the last block drain concurrently with the
            # final snapshots; the final slot goes through ONE shared tile
            # and a single DMA, so the tail pays one issue pipeline instead
            # of two serialized on the SP sequencer (DMA cannot read PSUM,
            # so the copies are unavoidable)
            fin = const.tile([NCHA * OUT, ncols], F32, tag="fin")
            for s_idx, st in enumerate(streams):
                drain_o(st, st.blks[kl], kl, 0, il)
                nc.vector.tensor_copy(fin[:, st.lo:st.lo + st.w], st.po)
            nc.sync.dma_start(
                scr.ap().rearrange("p (t n) -> p t n", n=ncols)[
                    :, t_total - 1:t_total, :],
                fin[:, :].rearrange("p (t n) -> p t n", n=ncols),
            )
        else:
            for k in range(nb):
                for s, st in enumerate(streams):
                    st.blk = rings[s].tile([128, tbuf * st.w], F32,
                                           tag=f"blk{s}")
                for i in range(tbuf):
                    slot = k * tbuf + i
                    for st in streams:
                        step(st, slot, k, i)
                for s, st in enumerate(streams):
                    drain(st, st.blk, k)

    nc.compile()
    if FUSE_WAITS:
        # Failure mid-pass leaves a correct program: each act's rewrite is
        # applied before its EventSemaphore is dropped (drops happen last),
        # so an aborted pass only leaves redundant waits behind.
        try:
            _fuse_act_waits(nc)
        except Exception:
            pass
    return nc


def prep_inputs(times, initial, Wi, bi, Wf0, bf0, Wf1, bf1, Wf2, bf2, Wf3, bf3,
                Wl, bl, t_total=T):
    """Host-side prep. Returns (shared map, per-core state list, accum_ok)."""
    f32 = np.float32
    times = np.asarray(times, f32)
    initial = np.asarray(initial, f32)
    Wi, bi = np.asarray(Wi, f32), np.asarray(bi, f32)
    W0, b0 = np.asarray(Wf0, f32), np.asarray(bf0, f32)
    W1, b1 = np.asarray(Wf1, f32), np.asarray(bf1, f32)
    W2, b2 = np.asarray(Wf2, f32), np.asarray(bf2, f32)
    W3, b3 = np.asarray(Wf3, f32), np.asarray(bf3, f32)
    Wl, bl = np.asarray(Wl, f32), np.asarray(bl, f32)

    GQ = (W3 @ W0).astype(f32)                            # [15, 15]
    GO = (W3 @ Wl).astype(f32)                            # [15, 8]
    cq = (b3 @ W0).astype(f32)                            # [15]
    co = (b3 @ Wl).astype(f32)                            # [8]

    dt = times[1:t_total] - times[:t_total - 1]           # [T-1]
    accum_ok = bool(np.all(dt == dt[0]))

    # initial state: p0 / o0 per batch row
    h0 = initial @ Wi + bi                                # [B, 32]
    p0 = (h0 @ W0).astype(f32)                            # [B, 15]
    o0 = (h0 @ Wl + bl).astype(f32)                       # [B, 8]

    shared = {}

    # ---- accum (stride-16) tensors, dt folded into G
    d0 = float(dt[0]) if accum_ok else 1.0
    w1a = np.zeros((128, 128), f32)
    w2a = np.zeros((128, 128), f32)
    ga = np.zeros((128, 128), f32)
    goa = np.zeros((128, CA), f32)
    bza = np.zeros((128, 4), f32)
    z2a = np.zeros((128, CA), f32)
    z2a[ONES_A, :] = 1.0
    for c in range(NCHA):
        r = 16 * c
        w1a[r:r + HH, r:r + HH] = W1
        w2a[r:r + HH, r:r + HH] = W2
        ga[r:r + HH, r:r + HH] = GQ * d0
        ga[ONES_A, r:r + HH] = cq * d0
        goa[r:r + HH, c * OUT:(c + 1) * OUT] = GO * d0
        goa[ONES_A, c * OUT:(c + 1) * OUT] = co * d0
        bza[r:r + HH, 0] = b0
        bza[r:r + HH, 1] = b1
        bza[r:r + HH, 2] = b2
    shared.update(w1a=w1a, w2a=w2a, ga=ga, goa=goa, bza=bza, z2a=z2a)

    # ---- non-accum (stride-32) tensors, per-step dt applied on DVE
    w1bd = np.zeros((128, 128), f32)
    w2bd = np.zeros((128, 128), f32)
    gbd = np.zeros((128, 128), f32)
    G23 = np.concatenate([GQ, GO], axis=1)                # [15, 23]
    gc23 = np.concatenate([cq, co])                       # [23]
    bzm = np.zeros((128, 4), f32)
    for c in range(NCH):
        r = 32 * c
        w1bd[r:r + HH, r:r + HH] = W1
        w2bd[r:r + HH, r:r + HH] = W2
        gbd[r:r + HH, r:r + HH + 8] = G23
        gbd[ONES_ROW, r:r + HH + 8] = gc23
        bzm[r:r + HH, 0] = b0
        bzm[r:r + HH, 1] = b1
        bzm[r:r + HH, 2] = b2
    z2init = np.zeros((128, 128), f32)
    z2init[ONES_ROW, :] = 1.0
    shared.update(w1bd=w1bd, w2bd=w2bd, gbd=gbd, bz=bzm, z2init=z2init,
                  dts=np.broadcast_to(dt, (128, t_total - 1)).copy())

    core_states = []
    for core in range(NCORES):
        rows0 = core * BSH
        # accum: s0p [128, 64] (p at 16c..16c+14), s0o [128, 64] (o at 8c+j)
        s0p = np.zeros((128, CA), f32)
        s0o = np.zeros((128, CA), f32)
        for c in range(NCHA):
            rows = slice(rows0 + c * CA, rows0 + (c + 1) * CA)
            s0p[16 * c:16 * c + HH, :] = p0[rows].T
            s0o[OUT * c:OUT * (c + 1), :] = o0[rows].T
        # non-accum: s32 [128, 128] (p at 32c.., o at 32c+15..)
        s32 = np.zeros((128, 128), f32)
        for c in range(NCH):
            rows = slice(rows0 + c * 128, rows0 + (c + 1) * 128)
            s32[32 * c:32 * c + HH, :] = p0[rows].T
            s32[32 * c + HH:32 * c + HH + 8, :] = o0[rows].T
        core_states.append({"s0p": s0p, "s0o": s0o, "s32": s32})

    return shared, core_states, accum_ok


def build_cst(shared, cs):
    """Assemble the consolidated per-core constant block [128, CST_COLS]."""
    cst = np.zeros((128, CST_COLS), np.float32)
    cst[:, CO_W1:CO_W1 + 128] = shared["w1a"]
    cst[:, CO_W2:CO_W2 + 128] = shared["w2a"]
    cst[:, CO_G:CO_G + 128] = shared["ga"]
    cst[:, CO_BZ:CO_BZ + 4] = shared["bza"]
    cst[:, CO_ID:CO_ID + 128] = np.eye(128, dtype=np.float32)
    cst[:, CO_S0:CO_S0 + CA] = cs["s0p"]
    cst[:, CO_GO:CO_GO + CA] = shared["goa"]
    cst[:, CO_S0O:CO_S0O + CA] = cs["s0o"]
    cst[:, CO_Z2:CO_Z2 + CA] = shared["z2a"]
    return cst


def unshard(scr_list, accum=True, t_total=T):
    """scratch [(c,o) rows, T*cols] per core -> full output [B, T, OUT]."""
    outs = []
    nch, ncols = (NCHA, CA) if accum else (NCH, 128)
    for scr in scr_list:
        s = scr.reshape(nch, OUT, t_total, ncols)         # [c, o, t, n]
        outs.append(np.ascontiguousarray(s.transpose(0, 3, 2, 1))
                    .reshape(BSH, t_total, OUT))
    return np.concatenate(outs, axis=0)


_CACHE = {}


def _get_program(t_total=T, tbuf=TBUF, nstream=NSTREAM, accum=True,
                 repeats=1):
    key = (t_total, tbuf, nstream, accum, repeats)
    if key not in _CACHE:
        _CACHE[key] = build_program(t_total, tbuf, nstream, accum, repeats)
    return _CACHE[key]


def kernel(**inputs) -> np.ndarray:
    from concourse.bass_utils import run_bass_kernel_spmd

    shared, core_states, accum_ok = prep_inputs(**inputs)
    nc = _get_program(accum=accum_ok)
    in_maps = []
    for core in range(NCORES):
        if accum_ok:
            m = {"cst": build_cst(shared, core_states[core])}
        else:
            m = {k: shared[k] for k in
                 ("w1bd", "w2bd", "gbd", "bz", "z2init", "dts")}
            m["s0"] = core_states[core]["s32"]
        in_maps.append(m)
    res = run_bass_kernel_spmd(nc, in_maps, core_ids=list(range(NCORES)))
    scr_list = [res.results[core]["oscr"] for core in range(NCORES)]
    return unshard(scr_list, accum=accum_ok)


# revision 52
# speedup vs baseline: 1.0004x; 1.0004x over previous
"""Trainium2 Bass kernel for a NeuralODE (forward-Euler scan over a tiny MLP).

Reference computation (per batch row x of `initial`):
    h0 = x @ Wi + bi                                  # [32]
    h_{t+1} = h_t + dt_t * f(h_t),  t = 0..T-2
    f(h) = tanh(tanh(tanh(h@W0+b0)@W1+b1)@W2+b2) @ W3 + b3
    out[t] = h_t @ Wl + bl                            # [8], t = 0..T-1

Device reformulation (exact in exact arithmetic): track the projected state
    p_t = W0^T h_t   (15-dim)     o_t = Wl^T h_t + bl   (8-dim = the output!)
since h_t only ever enters through W0 (layer 0) and Wl (readout):
    z  = tanh(p + b0); z = tanh(z@W1+b1); z2 = tanh(z@W2+b2)
    p += dt * (z2 @ (W3@W0) + b3@W0)
    o += dt * (z2 @ (W3@Wl) + b3@Wl)
This removes the h->z matmul and the separate trajectory projection pass:
the o-part of the state IS the output trajectory.

The scan is latency-bound: each step is a 6-hop cross-engine cycle
(act->mm->act->mm->act->mmG) whose per-hop cost is dominated by fixed
constants (act SBUF access 2x185ns, PE pipeline 173ns, sem hops ~50ns), so
the layout minimizes the per-instruction column width on the critical acts:

Accum layout (constant dt, the harness case): stride-16, 8 chunks.
  512 batch rows per core = 8 chunks x 64 batch columns.
  p-state tile [128 part, 64 cols]: chunk c occupies partitions 16c..16c+14;
  row 127 is the constant-1 bias row of z2 (chunk 7's spare). The o-state
  lives separately in po [64, 8c+j rows, cols], accumulated in its own PSUM
  bank per stream (PSUM start=True resets at bank granularity, so sharing a
  bank across streams would wipe earlier streams' o0 seeds).
  Weights are 128x128 / 128x64 block-diagonal matrices (host-assembled):
  W1bd/W2bd blocks [15,15] at (16c,16c); Gbd blocks [15,15] = dt*(W3@W0) at
  (16c,16c) plus row 127 = dt*(b3@W0) per chunk; Gbdo [16c+i, 8c+j] =
  dt*(W3@Wl)[i,j] plus row 127 = dt*(b3@Wl).
  Step: act0 -> mm1 -> act1 -> mm2 -> act2 -> mmG (accumulates pg in PSUM,
  start=False) + mmGo (accumulates po off-chain); a DVE copy snapshots
  o_{t-1} into a [64, TBUF*w] SBUF ring which DMA-drains to DRAM scratch
  [64, T*64]; the host transposes scratch (c,o;t,n) -> out[c*64+n, t, o].

  All per-core constants ship as ONE dram tensor / ONE dma_start (each
  dma_start costs ~650ns of serialized SP-sequencer issue time; the former
  11-DMA preamble burned ~7us before the first matmul).

Non-accum fallback (variable dt; unused by the harness): stride-32,
4 chunks of 128 cols, full 23-row state in SBUF ring blocks, DVE
scalar_tensor_tensor update with per-step dt from SBUF.

Post-compile, _fuse_act_waits attaches each act's spilled cross-engine wait
directly to the instruction (TRN2 allows 1 wait per instruction; the
scheduler spills the second onto a standalone EventSemaphore, which
serializes sem-fire -> SEQ-decode -> engine on the critical chain).
"""

from contextlib import ExitStack

import numpy as np

B, T = 4096, 1000
INIT_DIM, HID, HH, OUT = 16, 32, 15, 8
NCORES = 8
BSH = B // NCORES          # 512 batch rows per core
NSTREAM = 2                # independent dependency chains per core
TBUF = 4                   # time slots per ring block (4 divides 1000)
FUSE_WAITS = True          # post-compile: attach spilled act waits

# ---- accum (stride-16) layout
NCHA = 8                   # chunks per core
CA = 64                    # batch cols per core (8 x 64 = 512)
ONES_A = 127               # z2 constant-one row (chunk 7 spare)
ACT_HI_A = 127             # act2 writes partitions [0, ACT_HI_A)
# column offsets inside the consolidated per-core constant tensor, ordered
# by when the first step needs each region: act0(1) needs only [s0p|bz],
# mm1/mm2(1) need [w1|w2] one hop later, the PSUM seeds + act2(1)'s ones-row
# need [id|s0o|z2] before hop 5, and mmG/mmGo(1) need [g|go] last. The
# regions ship as four DMAs (three pipelined on SP, one on the Activation
# queue) so the chain starts on the small first transfer instead of
# stalling ~1us on one monolithic load.
CO_ID, CO_S0, CO_S0O, CO_BZ = 0, 128, 192, 256
CO_W1, CO_W2 = 260, 388
CO_G, CO_GO, CO_Z2 = 516, 644, 708
CO_D1, CO_D2 = 260, 516
CST_COLS = 772

# ---- non-accum (stride-32) layout
NCH = 4
ONES_ROW = 124
ACT_HI = 111


def _fuse_act_waits(nc):
    """Merge each Act-engine standalone EventSemaphore into the following
    Activation instruction.

    Every per-step act carries (a) a same-engine ordering wait
    S[Activation]>=k (WAW on its z output vs the previous step's act,
    guaranteed anyway by in-order engine execution and FIFO SBUF writes)
    and (b) a spilled cross-engine data wait. Attaching (b) directly to
    the act lets the sequencer pre-decode and queue it at the engine, so
    the sem fires straight into execution (~70-90ns per critical hop).
    """
    from concourse import mybir

    fn = nc.m.functions[0]
    for blk in fn.blocks:
        insts = blk.instructions
        drop = []
        pending = None          # (index, EventSemaphore) awaiting its act
        for idx, ins in enumerate(insts):
            if not str(ins.engine).endswith("Activation"):
                continue
            op = ins.opcode
            if op == "EventSemaphore":
                si = ins.sync_info
                if (pending is None and si is not None
                        and len(si.on_wait) == 1 and not si.on_update
                        and not si.on_wait[0].ant_name.startswith("Activation")):
                    pending = (idx, ins)
                else:
                    pending = None
            elif op == "Activation" and pending is not None:
                asi = ins.sync_info
                psi = pending[1].sync_info
                if (asi is not None and len(asi.on_wait) == 1
                        and asi.on_wait[0].ant_name.startswith("Activation")):
                    ins.sync_info = mybir.SyncInfo(
                        on_wait=[psi.on_wait[0]],
                        on_update=list(asi.on_update))
                    drop.append(pending[0])
                pending = None
            else:
                pending = None
        for idx in reversed(drop):
            del insts[idx]


def build_program(t_total=T, tbuf=TBUF, nstream=NSTREAM, accum=True,
                  repeats=1):
    """Build + compile the per-core Bass program (SPMD: same on all cores).

    `nstream` independent dependency chains, each covering a disjoint
    column-slice of the batch, interleave on the engines to hide the
    per-step cross-engine latency (the serial chain is ~2us/step).
    """
    import concourse.tile as tile
    from concourse import bacc, mybir

    F32 = mybir.dt.float32
    Tanh = mybir.ActivationFunctionType.Tanh

    nc = bacc.Bacc("TRN2", target_bir_lowering=False, debug=False)

    ncols = CA if accum else 128
    scr_rows = NCHA * OUT if accum else NCH * OUT
    nb = t_total // tbuf
    assert nb * tbuf == t_total
    base = ncols // nstream
    ws = [base + (1 if s < ncols % nstream else 0) for s in range(nstream)]

    scr = nc.dram_tensor("oscr", [scr_rows, t_total * ncols], F32,
                         kind="ExternalOutput")
    if accum:
        cst = nc.dram_tensor("cst", [128, CST_COLS], F32,
                             kind="ExternalInput")
    else:
        s0 = nc.dram_tensor("s0", [128, 128], F32, kind="ExternalInput")
        w1 = nc.dram_tensor("w1bd", [128, 128], F32, kind="ExternalInput")
        w2 = nc.dram_tensor("w2bd", [128, 128], F32, kind="ExternalInput")
        gm = nc.dram_tensor("gbd", [128, 128], F32, kind="ExternalInput")
        bzt = nc.dram_tensor("bz", [128, 4], F32, kind="ExternalInput")
        z2i = nc.dram_tensor("z2init", [128, 128], F32, kind="ExternalInput")
        dts = nc.dram_tensor("dts", [128, t_total - 1], F32,
                             kind="ExternalInput")

    with tile.TileContext(nc) as tc, ExitStack() as ctx:
        const = ctx.enter_context(tc.tile_pool(name="const", bufs=1))
        rings = [ctx.enter_context(tc.tile_pool(name=f"ring{s}", bufs=2))
                 for s in range(nstream)]
        psum = ctx.enter_context(tc.tile_pool(name="psum", bufs=1, space="PSUM"))

        if accum:
            cst_sb = const.tile([128, CST_COLS], F32, tag="cst")
            # D1 = [id|s0p|s0o|bz]: gates the PSUM seeds (which the
            # scheduler hoists to the PE front) and act0(1). D2 = [w1|w2]
            # pipelines right behind on the same SP queue, landing before
            # mm1(1) needs it. D3 = [g|go|z2] rides the Activation queue,
            # gating only hop >=3 of step 1.
            nc.sync.dma_start(cst_sb[:, 0:CO_D1], cst.ap()[:, 0:CO_D1])
            nc.gpsimd.dma_start(cst_sb[:, CO_D1:CO_D2],
                                cst.ap()[:, CO_D1:CO_D2])
            nc.scalar.dma_start(cst_sb[:, CO_D2:CST_COLS],
                                cst.ap()[:, CO_D2:CST_COLS])
            _c = cst_sb[:]
            w1_sb = _c[:, CO_W1:CO_W1 + 128]
            w2_sb = _c[:, CO_W2:CO_W2 + 128]
            g_sb = _c[:, CO_G:CO_G + 128]
            bz_sb = _c[:, CO_BZ:CO_BZ + 4]
            id_sb = _c[:, CO_ID:CO_ID + 128]
            s0_sb = _c[:, CO_S0:CO_S0 + CA]
            go_sb = _c[:, CO_GO:CO_GO + CA]
            s0o_sb = _c[:, CO_S0O:CO_S0O + CA]
            act_hi = ACT_HI_A
        else:
            w1t = const.tile([128, 128], F32, tag="w1")
            w2t = const.tile([128, 128], F32, tag="w2")
            gt = const.tile([128, 128], F32, tag="g")
            bzs = const.tile([128, 4], F32, tag="bz")
            dts_sb = const.tile([128, t_total - 1], F32, tag="dts")
            nc.sync.dma_start(w1t[:], w1.ap())
            nc.sync.dma_start(w2t[:], w2.ap())
            nc.sync.dma_start(gt[:], gm.ap())
            nc.sync.dma_start(bzs[:], bzt.ap())
            nc.sync.dma_start(dts_sb[:], dts.ap())
            w1_sb, w2_sb, g_sb, bz_sb = w1t[:], w2t[:], gt[:], bzs[:]
            act_hi = ACT_HI

        class Stream:
            pass

        # nstream=2 fits 4 private PSUM banks per stream. For nstream>=3,
        # merge p1/p2 into one transient bank per stream (mm2's start=True
        # reset of the bank is chain-ordered behind act1's read of p1) and
        # share a single po bank across streams, seeded by ONE start=True
        # matmul over all columns (per-stream seeds would wipe each other:
        # PSUM start=True resets has_written at bank granularity).
        share_psum = nstream >= 3 and accum
        po_all = None
        if share_psum:
            po_all = psum.tile([NCHA * OUT, ncols], F32, tag="po",
                               name="po_all")
            nc.tensor.matmul(po_all[:], id_sb[0:NCHA * OUT, 0:NCHA * OUT],
                             s0o_sb[0:NCHA * OUT, 0:ncols],
                             start=True, stop=False, skip_group_check=True)

        streams = []
        lo = 0
        for s in range(nstream):
            w = ws[s]
            st = Stream()
            st.lo, st.w = lo, w
            lo += w
            st.z0 = const.tile([128, w], F32, tag=f"z0_{s}")
            st.z1 = const.tile([128, w], F32, tag=f"z1_{s}")
            st.p1 = psum.tile([128, w], F32, tag=f"p1_{s}")
            st.p2 = st.p1 if share_psum else psum.tile(
                [128, w], F32, tag=f"p2_{s}", name=f"p2_{s}")
            st.pg = psum.tile([128, w], F32, tag=f"pg_{s}")
            st.prev = None
            st.blk = None
            if accum:
                # z2 lives inside the consolidated const tile: its ones-row
                # (and junk-row zeros) arrive with the single cst DMA; act2
                # overwrites rows [0, ACT_HI_A) each step.
                st.z2 = _c[:, CO_Z2 + st.lo:CO_Z2 + st.lo + w]
                # pg/po PSUM seeds are emitted inside step 1 (see
                # step_accum): the seeds wait on the cold DMA, and emitting
                # them here would head-of-line-block step 1's mm1/mm2 on the
                # in-order PE queue
                if share_psum:
                    st.po = po_all[:, st.lo:st.lo + w]
                else:
                    po = psum.tile([NCHA * OUT, w], F32, tag=f"po_{s}",
                                   name=f"po_{s}")
                    st.po = po[:]
            else:
                z2t = const.tile([128, w], F32, tag=f"z2_{s}",
                                 name=f"z2_{s}")
                st.z2 = z2t[:]
                nc.sync.dma_start(st.z2, z2i.ap()[:, st.lo:st.lo + w])
            streams.append(st)

        def step_accum(st, slot, blks):
            """Emit the chain producing state s_{slot}; also emit the
            snapshot copy of s_{slot-1} mid-emission so program order puts
            the next chain's act0 (a co-reader of the accumulator bank)
            ahead of the copy."""
            k1, i1 = divmod(slot - 1, tbuf)
            w = st.w
            prev_cur = st.blks[k1][:, i1 * w:(i1 + 1) * w]
            # slot 1 reads q0 straight from the SBUF constant block, so the
            # chain starts as soon as the hot DMA lands; the PSUM seed
            # matmuls only have to finish before mmG(1), five hops later
            q_in = s0_sb[:, st.lo:st.lo + w] if slot == 1 else st.pg[:]
            nc.scalar.activation(st.z0[:], q_in, Tanh, bias=bz_sb[:, 0:1])
            nc.tensor.matmul(st.p1[:], w1_sb, st.z0[:],
                             start=True, stop=True)
            # snapshot o_{slot-1} into the output ring (off the critical
            # path; at slot 1 it moves below the PSUM seeds it reads)
            if slot != 1:
                nc.vector.tensor_copy(prev_cur, st.po)
            nc.scalar.activation(st.z1[:], st.p1[:], Tanh, bias=bz_sb[:, 1:2])
            nc.tensor.matmul(st.p2[:], w2_sb, st.z1[:],
                             start=True, stop=True)
            if slot == 1:
                # PSUM seeds, placed after mm1/mm2 so their cold-DMA wait
                # cannot head-of-line-block the chain on the in-order PE,
                # and before the o_0 snapshot and accumulating mmG/mmGo.
                # The identity matmuls set the banks' has_written bits.
                nc.tensor.matmul(st.pg[:], id_sb,
                                 s0_sb[:, st.lo:st.lo + w],
                                 start=True, stop=False, skip_group_check=True)
                if not share_psum:
                    nc.tensor.matmul(
                        st.po, id_sb[0:NCHA * OUT, 0:NCHA * OUT],
                        s0o_sb[0:NCHA * OUT, st.lo:st.lo + w],
                        start=True, stop=False, skip_group_check=True)
                nc.vector.tensor_copy(prev_cur, st.po)
            nc.scalar.activation(
                st.z2[0:act_hi, :], st.p2[0:act_hi, :], Tanh,
                bias=bz_sb[0:act_hi, 2:3],
            )
            # p += (dt*G_p)^T z2 and o += (dt*G_o)^T z2, accumulated by the PE
            # (the final slot's q-update feeds nothing: q_{T} is never read,
            # so skip its mmG and let the last mmGo fire sooner)
            if slot < t_total - 1:
                nc.tensor.matmul(st.pg[:], g_sb, st.z2,
                                 start=False, stop=False,
                                 skip_group_check=True)
            nc.tensor.matmul(st.po, go_sb, st.z2,
                             start=False, stop=False, skip_group_check=True)

        def step(st, slot, k, i):
            w = st.w
            cur = st.blk[:, i * w:(i + 1) * w]
            if slot == 0:
                nc.sync.dma_start(cur, s0.ap()[:, st.lo:st.lo + w])
                st.prev = cur
                return
            nc.scalar.activation(st.z0[:], st.prev, Tanh, bias=bz_sb[:, 0:1])
            nc.tensor.matmul(st.p1[:], w1_sb, st.z0[:], start=True, stop=True)
            nc.scalar.activation(st.z1[:], st.p1[:], Tanh, bias=bz_sb[:, 1:2])
            nc.tensor.matmul(st.p2[:], w2_sb, st.z1[:], start=True, stop=True)
            nc.scalar.activation(
                st.z2[0:act_hi, :], st.p2[0:act_hi, :], Tanh,
                bias=bz_sb[0:act_hi, 2:3],
            )
            nc.tensor.matmul(st.pg[:], g_sb, st.z2, start=True, stop=True)
            nc.vector.scalar_tensor_tensor(
                cur, st.pg[:], dts_sb[:, slot - 1:slot], st.prev,
                mybir.AluOpType.mult, mybir.AluOpType.add,
            )
            st.prev = cur

        def drain(st, blk, k):
            # block k's o-rows -> DRAM scratch (non-accum, 32-stride state)
            for c in range(NCH):
                nc.sync.dma_start(
                    scr.ap().rearrange("p (t n) -> p t n", n=ncols)[
                        c * 8:(c + 1) * 8, k * tbuf:(k + 1) * tbuf,
                        st.lo:st.lo + st.w],
                    blk[32 * c + 15:32 * c + 23, :].rearrange(
                        "p (t n) -> p t n", n=st.w),
                )

        def drain_o(st, blk, k, i0=0, i1=None):
            # accum mode: blk is already [64=(c,o), tbuf*w], matching scr rows
            i1 = tbuf if i1 is None else i1
            nc.sync.dma_start(
                scr.ap().rearrange("p (t n) -> p t n", n=ncols)[
                    :, k * tbuf + i0:k * tbuf + i1, st.lo:st.lo + st.w],
                blk[:, i0 * st.w:i1 * st.w].rearrange(
                    "p (t n) -> p t n", n=st.w),
            )

        if accum:
            for s, st in enumerate(streams):
                st.blks = {}

            def get_blk(st, s_idx, k):
                if k not in st.blks:
                    st.blks[k] = rings[s_idx].tile(
                        [NCHA * OUT, tbuf * st.w], F32, tag=f"blk{s_idx}",
                        name=f"blk{s_idx}_{k}")
                return st.blks[k]

            for rep in range(repeats):
                if rep:
                    for st in streams:   # fresh ring tiles each repeat
                        st.blks = {}
                for slot in range(1, t_total):
                    k = slot // tbuf
                    for s_idx, st in enumerate(streams):
                        get_blk(st, s_idx, (slot - 1) // tbuf)
                        get_blk(st, s_idx, k)
                        step_accum(st, slot, st.blks)
                        if slot % tbuf == 0:
                            drain_o(st, st.blks[k - 1], k - 1)
            kl, il = divmod(t_total - 1, tbuf)
            # completed slots of the last block drain concurrently with the
            # final snapshots; the final slot goes through ONE shared tile
            # and a single DMA, so the tail pays one issue pipeline instead
            # of two serialized on the SP sequencer (DMA cannot read PSUM,
            # so the copies are unavoidable)
            fin = const.tile([NCHA * OUT, ncols], F32, tag="fin")
            for s_idx, st in enumerate(streams):
                drain_o(st, st.blks[kl], kl, 0, il)
                nc.vector.tensor_copy(fin[:, st.lo:st.lo + st.w], st.po)
            nc.sync.dma_start(
                scr.ap().rearrange("p (t n) -> p t n", n=ncols)[
                    :, t_total - 1:t_total, :],
                fin[:, :].rearrange("p (t n) -> p t n", n=ncols),
            )
        else:
            for k in range(nb):
                for s, st in enumerate(streams):
                    st.blk = rings[s].tile([128, tbuf * st.w], F32,
                                           tag=f"blk{s}")
                for i in range(tbuf):
                    slot = k * tbuf + i
                    for st in streams:
                        step(st, slot, k, i)
                for s, st in enumerate(streams):
                    drain(st, st.blk, k)

    nc.compile()
    if FUSE_WAITS:
        # Failure mid-pass leaves a correct program: each act's rewrite is
        # applied before its EventSemaphore is dropped (drops happen last),
        # so an aborted pass only leaves redundant waits behind.
        try:
            _fuse_act_waits(nc)
        except Exception:
            pass
    return nc


def prep_inputs(times, initial, Wi, bi, Wf0, bf0, Wf1, bf1, Wf2, bf2, Wf3, bf3,
                Wl, bl, t_total=T):
    """Host-side prep. Returns (shared map, per-core state list, accum_ok)."""
    f32 = np.float32
    times = np.asarray(times, f32)
    initial = np.asarray(initial, f32)
    Wi, bi = np.asarray(Wi, f32), np.asarray(bi, f32)
    W0, b0 = np.asarray(Wf0, f32), np.asarray(bf0, f32)
    W1, b1 = np.asarray(Wf1, f32), np.asarray(bf1, f32)
    W2, b2 = np.asarray(Wf2, f32), np.asarray(bf2, f32)
    W3, b3 = np.asarray(Wf3, f32), np.asarray(bf3, f32)
    Wl, bl = np.asarray(Wl, f32), np.asarray(bl, f32)

    GQ = (W3 @ W0).astype(f32)                            # [15, 15]
    GO = (W3 @ Wl).astype(f32)                            # [15, 8]
    cq = (b3 @ W0).astype(f32)                            # [15]
    co = (b3 @ Wl).astype(f32)                            # [8]

    dt = times[1:t_total] - times[:t_total - 1]           # [T-1]
    accum_ok = bool(np.all(dt == dt[0]))

    # initial state: p0 / o0 per batch row
    h0 = initial @ Wi + bi                                # [B, 32]
    p0 = (h0 @ W0).astype(f32)                            # [B, 15]
    o0 = (h0 @ Wl + bl).astype(f32)                       # [B, 8]

    shared = {}

    # ---- accum (stride-16) tensors, dt folded into G
    d0 = float(dt[0]) if accum_ok else 1.0
    w1a = np.zeros((128, 128), f32)
    w2a = np.zeros((128, 128), f32)
    ga = np.zeros((128, 128), f32)
    goa = np.zeros((128, CA), f32)
    bza = np.zeros((128, 4), f32)
    z2a = np.zeros((128, CA), f32)
    z2a[ONES_A, :] = 1.0
    for c in range(NCHA):
        r = 16 * c
        w1a[r:r + HH, r:r + HH] = W1
        w2a[r:r + HH, r:r + HH] = W2
        ga[r:r + HH, r:r + HH] = GQ * d0
        ga[ONES_A, r:r + HH] = cq * d0
        goa[r:r + HH, c * OUT:(c + 1) * OUT] = GO * d0
        goa[ONES_A, c * OUT:(c + 1) * OUT] = co * d0
        bza[r:r + HH, 0] = b0
        bza[r:r + HH, 1] = b1
        bza[r:r + HH, 2] = b2
    shared.update(w1a=w1a, w2a=w2a, ga=ga, goa=goa, bza=bza, z2a=z2a)

    # ---- non-accum (stride-32) tensors, per-step dt applied on DVE
    w1bd = np.zeros((128, 128), f32)
    w2bd = np.zeros((128, 128), f32)
    gbd = np.zeros((128, 128), f32)
    G23 = np.concatenate([GQ, GO], axis=1)                # [15, 23]
    gc23 = np.concatenate([cq, co])                       # [23]
    bzm = np.zeros((128, 4), f32)
    for c in range(NCH):
        r = 32 * c
        w1bd[r:r + HH, r:r + HH] = W1
        w2bd[r:r + HH, r:r + HH] = W2
        gbd[r:r + HH, r:r + HH + 8] = G23
        gbd[ONES_ROW, r:r + HH + 8] = gc23
        bzm[r:r + HH, 0] = b0
        bzm[r:r + HH, 1] = b1
        bzm[r:r + HH, 2] = b2
    z2init = np.zeros((128, 128), f32)
    z2init[ONES_ROW, :] = 1.0
    shared.update(w1bd=w1bd, w2bd=w2bd, gbd=gbd, bz=bzm, z2init=z2init,
                  dts=np.broadcast_to(dt, (128, t_total - 1)).copy())

    core_states = []
    for core in range(NCORES):
        rows0 = core * BSH
        # accum: s0p [128, 64] (p at 16c..16c+14), s0o [128, 64] (o at 8c+j)
        s0p = np.zeros((128, CA), f32)
        s0o = np.zeros((128, CA), f32)
        for c in range(NCHA):
            rows = slice(rows0 + c * CA, rows0 + (c + 1) * CA)
            s0p[16 * c:16 * c + HH, :] = p0[rows].T
            s0o[OUT * c:OUT * (c + 1), :] = o0[rows].T
        # non-accum: s32 [128, 128] (p at 32c.., o at 32c+15..)
        s32 = np.zeros((128, 128), f32)
        for c in range(NCH):
            rows = slice(rows0 + c * 128, rows0 + (c + 1) * 128)
            s32[32 * c:32 * c + HH, :] = p0[rows].T
            s32[32 * c + HH:32 * c + HH + 8, :] = o0[rows].T
        core_states.append({"s0p": s0p, "s0o": s0o, "s32": s32})

    return shared, core_states, accum_ok


def build_cst(shared, cs):
    """Assemble the consolidated per-core constant block [128, CST_COLS]."""
    cst = np.zeros((128, CST_COLS), np.float32)
    cst[:, CO_W1:CO_W1 + 128] = shared["w1a"]
    cst[:, CO_W2:CO_W2 + 128] = shared["w2a"]
    cst[:, CO_G:CO_G + 128] = shared["ga"]
    cst[:, CO_BZ:CO_BZ + 4] = shared["bza"]
    cst[:, CO_ID:CO_ID + 128] = np.eye(128, dtype=np.float32)
    cst[:, CO_S0:CO_S0 + CA] = cs["s0p"]
    cst[:, CO_GO:CO_GO + CA] = shared["goa"]
    cst[:, CO_S0O:CO_S0O + CA] = cs["s0o"]
    cst[:, CO_Z2:CO_Z2 + CA] = shared["z2a"]
    return cst


def unshard(scr_list, accum=True, t_total=T):
    """scratch [(c,o) rows, T*cols] per core -> full output [B, T, OUT]."""
    outs = []
    nch, ncols = (NCHA, CA) if accum else (NCH, 128)
    for scr in scr_list:
        s = scr.reshape(nch, OUT, t_total, ncols)         # [c, o, t, n]
        outs.append(np.ascontiguousarray(s.transpose(0, 3, 2, 1))
                    .reshape(BSH, t_total, OUT))
    return np.concatenate(outs, axis=0)


_CACHE = {}


def _get_program(t_total=T, tbuf=TBUF, nstream=NSTREAM, accum=True,
                 repeats=1):
    key = (t_total, tbuf, nstream, accum, repeats)
    if key not in _CACHE:
        _CACHE[key] = build_program(t_total, tbuf, nstream, accum, repeats)
    return _CACHE[key]


def kernel(**inputs) -> np.ndarray:
    from concourse.bass_utils import run_bass_kernel_spmd

    shared, core_states, accum_ok = prep_inputs(**inputs)
    nc = _get_program(accum=accum_ok)
    in_maps = []
    for core in range(NCORES):
        if accum_ok:
            m = {"cst": build_cst(shared, core_states[core])}
        else:
            m = {k: shared[k] for k in
                 ("w1bd", "w2bd", "gbd", "bz", "z2init", "dts")}
            m["s0"] = core_states[core]["s32"]
        in_maps.append(m)
    res = run_bass_kernel_spmd(nc, in_maps, core_ids=list(range(NCORES)))
    scr_list = [res.results[core]["oscr"] for core in range(NCORES)]
    return unshard(scr_list, accum=accum_ok)


# revision 55
# speedup vs baseline: 1.0004x; 1.0000x over previous
"""Trainium2 Bass kernel for a NeuralODE (forward-Euler scan over a tiny MLP).

Reference computation (per batch row x of `initial`):
    h0 = x @ Wi + bi                                  # [32]
    h_{t+1} = h_t + dt_t * f(h_t),  t = 0..T-2
    f(h) = tanh(tanh(tanh(h@W0+b0)@W1+b1)@W2+b2) @ W3 + b3
    out[t] = h_t @ Wl + bl                            # [8], t = 0..T-1

Device reformulation (exact in exact arithmetic): track the projected state
    p_t = W0^T h_t   (15-dim)     o_t = Wl^T h_t + bl   (8-dim = the output!)
since h_t only ever enters through W0 (layer 0) and Wl (readout):
    z  = tanh(p + b0); z = tanh(z@W1+b1); z2 = tanh(z@W2+b2)
    p += dt * (z2 @ (W3@W0) + b3@W0)
    o += dt * (z2 @ (W3@Wl) + b3@Wl)
This removes the h->z matmul and the separate trajectory projection pass:
the o-part of the state IS the output trajectory.

The scan is latency-bound: each step is a 6-hop cross-engine cycle
(act->mm->act->mm->act->mmG) whose per-hop cost is dominated by fixed
constants (act SBUF access 2x185ns, PE pipeline 173ns, sem hops ~50ns), so
the layout minimizes the per-instruction column width on the critical acts:

Accum layout (constant dt, the harness case): stride-16, 8 chunks.
  512 batch rows per core = 8 chunks x 64 batch columns.
  p-state tile [128 part, 64 cols]: chunk c occupies partitions 16c..16c+14;
  row 127 is the constant-1 bias row of z2 (chunk 7's spare). The o-state
  lives separately in po [64, 8c+j rows, cols], accumulated in its own PSUM
  bank per stream (PSUM start=True resets at bank granularity, so sharing a
  bank across streams would wipe earlier streams' o0 seeds).
  Weights are 128x128 / 128x64 block-diagonal matrices (host-assembled):
  W1bd/W2bd blocks [15,15] at (16c,16c); Gbd blocks [15,15] = dt*(W3@W0) at
  (16c,16c) plus row 127 = dt*(b3@W0) per chunk; Gbdo [16c+i, 8c+j] =
  dt*(W3@Wl)[i,j] plus row 127 = dt*(b3@Wl).
  Step: act0 -> mm1 -> act1 -> mm2 -> act2 -> mmG (accumulates pg in PSUM,
  start=False) + mmGo (accumulates po off-chain); a DVE copy snapshots
  o_{t-1} into a [64, TBUF*w] SBUF ring which DMA-drains to DRAM scratch
  [64, T*64]; the host transposes scratch (c,o;t,n) -> out[c*64+n, t, o].

  All per-core constants ship as ONE dram tensor / ONE dma_start (each
  dma_start costs ~650ns of serialized SP-sequencer issue time; the former
  11-DMA preamble burned ~7us before the first matmul).

Non-accum fallback (variable dt; unused by the harness): stride-32,
4 chunks of 128 cols, full 23-row state in SBUF ring blocks, DVE
scalar_tensor_tensor update with per-step dt from SBUF.

Post-compile, _fuse_act_waits attaches each act's spilled cross-engine wait
directly to the instruction (TRN2 allows 1 wait per instruction; the
scheduler spills the second onto a standalone EventSemaphore, which
serializes sem-fire -> SEQ-decode -> engine on the critical chain).
"""

from contextlib import ExitStack

import numpy as np

B, T = 4096, 1000
INIT_DIM, HID, HH, OUT = 16, 32, 15, 8
NCORES = 8
BSH = B // NCORES          # 512 batch rows per core
NSTREAM = 2                # independent dependency chains per core
TBUF = 4                   # time slots per ring block (4 divides 1000)
FUSE_WAITS = True          # post-compile: attach spilled act waits

# ---- accum (stride-16) layout
NCHA = 8                   # chunks per core
CA = 64                    # batch cols per core (8 x 64 = 512)
ONES_A = 127               # z2 constant-one row (chunk 7 spare)
ACT_HI_A = 127             # act2 writes partitions [0, ACT_HI_A)
# column offsets inside the consolidated per-core constant tensor, ordered
# by when the first step needs each region: act0(1) needs only [s0p|bz],
# mm1/mm2(1) need [w1|w2] one hop later, the PSUM seeds + act2(1)'s ones-row
# need [id|s0o|z2] before hop 5, and mmG/mmGo(1) need [g|go] last. The
# regions ship as four DMAs (three pipelined on SP, one on the Activation
# queue) so the chain starts on the small first transfer instead of
# stalling ~1us on one monolithic load.
CO_ID, CO_S0, CO_BZ = 0, 128, 192
CO_W1, CO_W2 = 196, 324
CO_G, CO_GO, CO_Z2 = 452, 580, 644
CO_D1, CO_D2 = 196, 452
CST_COLS = 708

# ---- non-accum (stride-32) layout
NCH = 4
ONES_ROW = 124
ACT_HI = 111


def _fuse_act_waits(nc):
    """Merge each Act-engine standalone EventSemaphore into the following
    Activation instruction.

    Every per-step act carries (a) a same-engine ordering wait
    S[Activation]>=k (WAW on its z output vs the previous step's act,
    guaranteed anyway by in-order engine execution and FIFO SBUF writes)
    and (b) a spilled cross-engine data wait. Attaching (b) directly to
    the act lets the sequencer pre-decode and queue it at the engine, so
    the sem fires straight into execution (~70-90ns per critical hop).
    """
    from concourse import mybir

    fn = nc.m.functions[0]
    for blk in fn.blocks:
        insts = blk.instructions
        drop = []
        pending = None          # (index, EventSemaphore) awaiting its act
        for idx, ins in enumerate(insts):
            if not str(ins.engine).endswith("Activation"):
                continue
            op = ins.opcode
            if op == "EventSemaphore":
                si = ins.sync_info
                if (pending is None and si is not None
                        and len(si.on_wait) == 1 and not si.on_update
                        and not si.on_wait[0].ant_name.startswith("Activation")):
                    pending = (idx, ins)
                else:
                    pending = None
            elif op == "Activation" and pending is not None:
                asi = ins.sync_info
                psi = pending[1].sync_info
                if (asi is not None and len(asi.on_wait) == 1
                        and asi.on_wait[0].ant_name.startswith("Activation")):
                    ins.sync_info = mybir.SyncInfo(
                        on_wait=[psi.on_wait[0]],
                        on_update=list(asi.on_update))
                    drop.append(pending[0])
                pending = None
            else:
                pending = None
        for idx in reversed(drop):
            del insts[idx]


def build_program(t_total=T, tbuf=TBUF, nstream=NSTREAM, accum=True,
                  repeats=1):
    """Build + compile the per-core Bass program (SPMD: same on all cores).

    `nstream` independent dependency chains, each covering a disjoint
    column-slice of the batch, interleave on the engines to hide the
    per-step cross-engine latency (the serial chain is ~2us/step).
    """
    import concourse.tile as tile
    from concourse import bacc, mybir

    F32 = mybir.dt.float32
    Tanh = mybir.ActivationFunctionType.Tanh

    nc = bacc.Bacc("TRN2", target_bir_lowering=False, debug=False)

    ncols = CA if accum else 128
    scr_rows = NCHA * OUT if accum else NCH * OUT
    nb = t_total // tbuf
    assert nb * tbuf == t_total
    base = ncols // nstream
    ws = [base + (1 if s < ncols % nstream else 0) for s in range(nstream)]

    scr = nc.dram_tensor("oscr", [scr_rows, t_total * ncols], F32,
                         kind="ExternalOutput")
    if accum:
        cst = nc.dram_tensor("cst", [128, CST_COLS], F32,
                             kind="ExternalInput")
    else:
        s0 = nc.dram_tensor("s0", [128, 128], F32, kind="ExternalInput")
        w1 = nc.dram_tensor("w1bd", [128, 128], F32, kind="ExternalInput")
        w2 = nc.dram_tensor("w2bd", [128, 128], F32, kind="ExternalInput")
        gm = nc.dram_tensor("gbd", [128, 128], F32, kind="ExternalInput")
        bzt = nc.dram_tensor("bz", [128, 4], F32, kind="ExternalInput")
        z2i = nc.dram_tensor("z2init", [128, 128], F32, kind="ExternalInput")
        dts = nc.dram_tensor("dts", [128, t_total - 1], F32,
                             kind="ExternalInput")

    with tile.TileContext(nc) as tc, ExitStack() as ctx:
        const = ctx.enter_context(tc.tile_pool(name="const", bufs=1))
        rings = [ctx.enter_context(tc.tile_pool(name=f"ring{s}", bufs=2))
                 for s in range(nstream)]
        psum = ctx.enter_context(tc.tile_pool(name="psum", bufs=1, space="PSUM"))

        if accum:
            cst_sb = const.tile([128, CST_COLS], F32, tag="cst")
            # D1 = [id|s0p|s0o|bz]: gates the PSUM seeds (which the
            # scheduler hoists to the PE front) and act0(1). D2 = [w1|w2]
            # pipelines right behind on the same SP queue, landing before
            # mm1(1) needs it. D3 = [g|go|z2] rides the Activation queue,
            # gating only hop >=3 of step 1.
            nc.sync.dma_start(cst_sb[:, 0:CO_D1], cst.ap()[:, 0:CO_D1])
            nc.gpsimd.dma_start(cst_sb[:, CO_D1:CO_D2],
                                cst.ap()[:, CO_D1:CO_D2])
            nc.scalar.dma_start(cst_sb[:, CO_D2:CST_COLS],
                                cst.ap()[:, CO_D2:CST_COLS])
            _c = cst_sb[:]
            w1_sb = _c[:, CO_W1:CO_W1 + 128]
            w2_sb = _c[:, CO_W2:CO_W2 + 128]
            g_sb = _c[:, CO_G:CO_G + 128]
            bz_sb = _c[:, CO_BZ:CO_BZ + 4]
            id_sb = _c[:, CO_ID:CO_ID + 128]
            s0_sb = _c[:, CO_S0:CO_S0 + CA]
            go_sb = _c[:, CO_GO:CO_GO + CA]
            act_hi = ACT_HI_A
        else:
            w1t = const.tile([128, 128], F32, tag="w1")
            w2t = const.tile([128, 128], F32, tag="w2")
            gt = const.tile([128, 128], F32, tag="g")
            bzs = const.tile([128, 4], F32, tag="bz")
            dts_sb = const.tile([128, t_total - 1], F32, tag="dts")
            nc.sync.dma_start(w1t[:], w1.ap())
            nc.sync.dma_start(w2t[:], w2.ap())
            nc.sync.dma_start(gt[:], gm.ap())
            nc.sync.dma_start(bzs[:], bzt.ap())
            nc.sync.dma_start(dts_sb[:], dts.ap())
            w1_sb, w2_sb, g_sb, bz_sb = w1t[:], w2t[:], gt[:], bzs[:]
            act_hi = ACT_HI

        class Stream:
            pass

        # nstream=2 fits 4 private PSUM banks per stream. For nstream>=3,
        # merge p1/p2 into one transient bank per stream (mm2's start=True
        # reset of the bank is chain-ordered behind act1's read of p1) and
        # share a single po bank across streams, seeded by ONE start=True
        # matmul over all columns (per-stream seeds would wipe each other:
        # PSUM start=True resets has_written at bank granularity).
        share_psum = nstream >= 3 and accum
        po_all = None
        if share_psum:
            po_all = psum.tile([NCHA * OUT, ncols], F32, tag="po",
                               name="po_all")

        streams = []
        lo = 0
        for s in range(nstream):
            w = ws[s]
            st = Stream()
            st.lo, st.w = lo, w
            lo += w
            st.z0 = const.tile([128, w], F32, tag=f"z0_{s}")
            st.z1 = const.tile([128, w], F32, tag=f"z1_{s}")
            st.p1 = psum.tile([128, w], F32, tag=f"p1_{s}")
            st.p2 = st.p1 if share_psum else psum.tile(
                [128, w], F32, tag=f"p2_{s}", name=f"p2_{s}")
            st.pg = psum.tile([128, w], F32, tag=f"pg_{s}")
            st.prev = None
            st.blk = None
            if accum:
                # z2 lives inside the consolidated const tile: its ones-row
                # (and junk-row zeros) arrive with the single cst DMA; act2
                # overwrites rows [0, ACT_HI_A) each step.
                st.z2 = _c[:, CO_Z2 + st.lo:CO_Z2 + st.lo + w]
                # pg/po PSUM seeds are emitted inside step 1 (see
                # step_accum): the seeds wait on the cold DMA, and emitting
                # them here would head-of-line-block step 1's mm1/mm2 on the
                # in-order PE queue
                if share_psum:
                    st.po = po_all[:, st.lo:st.lo + w]
                else:
                    po = psum.tile([NCHA * OUT, w], F32, tag=f"po_{s}",
                                   name=f"po_{s}")
                    st.po = po[:]
            else:
                z2t = const.tile([128, w], F32, tag=f"z2_{s}",
                                 name=f"z2_{s}")
                st.z2 = z2t[:]
                nc.sync.dma_start(st.z2, z2i.ap()[:, st.lo:st.lo + w])
            streams.append(st)

        def step_accum(st, slot, blks):
            """Emit the chain producing state s_{slot}; also emit the
            snapshot copy of s_{slot-1} mid-emission so program order puts
            the next chain's act0 (a co-reader of the accumulator bank)
            ahead of the copy."""
            k1, i1 = divmod(slot - 1, tbuf)
            w = st.w
            prev_cur = st.blks[k1][:, i1 * w:(i1 + 1) * w]
            # slot 1 reads q0 straight from the SBUF constant block, so the
            # chain starts as soon as the hot DMA lands; the PSUM seed
            # matmuls only have to finish before mmG(1), five hops later
            q_in = s0_sb[:, st.lo:st.lo + w] if slot == 1 else st.pg[:]
            nc.scalar.activation(st.z0[:], q_in, Tanh, bias=bz_sb[:, 0:1])
            nc.tensor.matmul(st.p1[:], w1_sb, st.z0[:],
                             start=True, stop=True)
            # snapshot o_{slot-1} into the output ring (off the critical
            # path; at slot 1 it moves below the PSUM seeds it reads)
            if slot != 1:
                nc.vector.tensor_copy(prev_cur, st.po)
            nc.scalar.activation(st.z1[:], st.p1[:], Tanh, bias=bz_sb[:, 1:2])
            nc.tensor.matmul(st.p2[:], w2_sb, st.z1[:],
                             start=True, stop=True)
            if slot == 1:
                # PSUM seeds, placed after mm1/mm2 so their cold-DMA wait
                # cannot head-of-line-block the chain on the in-order PE,
                # and before the o_0 snapshot and accumulating mmG/mmGo.
                # The identity matmuls set the banks' has_written bits.
                nc.tensor.matmul(st.pg[:], id_sb,
                                 s0_sb[:, st.lo:st.lo + w],
                                 start=True, stop=False, skip_group_check=True)
                # po is NOT seeded: mmGo(1) opens the group with start=True
                # and the device emits the o-INCREMENT trajectory; the host
                # adds the constant o0 during unshard (o is terminal, so the
                # changed rounding order does not amplify). t=0's ring slot
                # stays unwritten garbage and is overwritten by the host.
                nc.vector.memset(prev_cur, 0.0)
            nc.scalar.activation(
                st.z2[0:act_hi, :], st.p2[0:act_hi, :], Tanh,
                bias=bz_sb[0:act_hi, 2:3],
            )
            # p += (dt*G_p)^T z2 and o += (dt*G_o)^T z2, accumulated by the PE
            # (the final slot's q-update feeds nothing: q_{T} is never read,
            # so skip its mmG and let the last mmGo fire sooner)
            if slot < t_total - 1:
                nc.tensor.matmul(st.pg[:], g_sb, st.z2,
                                 start=False, stop=False,
                                 skip_group_check=True)
            nc.tensor.matmul(st.po, go_sb, st.z2,
                             start=(slot == 1), stop=False,
                             skip_group_check=True)

        def step(st, slot, k, i):
            w = st.w
            cur = st.blk[:, i * w:(i + 1) * w]
            if slot == 0:
                nc.sync.dma_start(cur, s0.ap()[:, st.lo:st.lo + w])
                st.prev = cur
                return
            nc.scalar.activation(st.z0[:], st.prev, Tanh, bias=bz_sb[:, 0:1])
            nc.tensor.matmul(st.p1[:], w1_sb, st.z0[:], start=True, stop=True)
            nc.scalar.activation(st.z1[:], st.p1[:], Tanh, bias=bz_sb[:, 1:2])
            nc.tensor.matmul(st.p2[:], w2_sb, st.z1[:], start=True, stop=True)
            nc.scalar.activation(
                st.z2[0:act_hi, :], st.p2[0:act_hi, :], Tanh,
                bias=bz_sb[0:act_hi, 2:3],
            )
            nc.tensor.matmul(st.pg[:], g_sb, st.z2, start=True, stop=True)
            nc.vector.scalar_tensor_tensor(
                cur, st.pg[:], dts_sb[:, slot - 1:slot], st.prev,
                mybir.AluOpType.mult, mybir.AluOpType.add,
            )
            st.prev = cur

        def drain(st, blk, k):
            # block k's o-rows -> DRAM scratch (non-accum, 32-stride state)
            for c in range(NCH):
                nc.sync.dma_start(
                    scr.ap().rearrange("p (t n) -> p t n", n=ncols)[
                        c * 8:(c + 1) * 8, k * tbuf:(k + 1) * tbuf,
                        st.lo:st.lo + st.w],
                    blk[32 * c + 15:32 * c + 23, :].rearrange(
                        "p (t n) -> p t n", n=st.w),
                )

        def drain_o(st, blk, k, i0=0, i1=None):
            # accum mode: blk is already [64=(c,o), tbuf*w], matching scr rows
            i1 = tbuf if i1 is None else i1
            nc.sync.dma_start(
                scr.ap().rearrange("p (t n) -> p t n", n=ncols)[
                    :, k * tbuf + i0:k * tbuf + i1, st.lo:st.lo + st.w],
                blk[:, i0 * st.w:i1 * st.w].rearrange(
                    "p (t n) -> p t n", n=st.w),
            )

        if accum:
            for s, st in enumerate(streams):
                st.blks = {}

            def get_blk(st, s_idx, k):
                if k not in st.blks:
                    st.blks[k] = rings[s_idx].tile(
                        [NCHA * OUT, tbuf * st.w], F32, tag=f"blk{s_idx}",
                        name=f"blk{s_idx}_{k}")
                return st.blks[k]

            for rep in range(repeats):
                if rep:
                    for st in streams:   # fresh ring tiles each repeat
                        st.blks = {}
                for slot in range(1, t_total):
                    k = slot // tbuf
                    for s_idx, st in enumerate(streams):
                        get_blk(st, s_idx, (slot - 1) // tbuf)
                        get_blk(st, s_idx, k)
                        step_accum(st, slot, st.blks)
                        if slot % tbuf == 0:
                            drain_o(st, st.blks[k - 1], k - 1)
            kl, il = divmod(t_total - 1, tbuf)
            # completed slots of the last block drain concurrently with the
            # final snapshots; the final slot goes through ONE shared tile
            # and a single DMA, so the tail pays one issue pipeline instead
            # of two serialized on the SP sequencer (DMA cannot read PSUM,
            # so the copies are unavoidable)
            fin = const.tile([NCHA * OUT, ncols], F32, tag="fin")
            for s_idx, st in enumerate(streams):
                drain_o(st, st.blks[kl], kl, 0, il)
                nc.vector.tensor_copy(fin[:, st.lo:st.lo + st.w], st.po)
            nc.sync.dma_start(
                scr.ap().rearrange("p (t n) -> p t n", n=ncols)[
                    :, t_total - 1:t_total, :],
                fin[:, :].rearrange("p (t n) -> p t n", n=ncols),
            )
        else:
            for k in range(nb):
                for s, st in enumerate(streams):
                    st.blk = rings[s].tile([128, tbuf * st.w], F32,
                                           tag=f"blk{s}")
                for i in range(tbuf):
                    slot = k * tbuf + i
                    for st in streams:
                        step(st, slot, k, i)
                for s, st in enumerate(streams):
                    drain(st, st.blk, k)

    nc.compile()
    if FUSE_WAITS:
        # Failure mid-pass leaves a correct program: each act's rewrite is
        # applied before its EventSemaphore is dropped (drops happen last),
        # so an aborted pass only leaves redundant waits behind.
        try:
            _fuse_act_waits(nc)
        except Exception:
            pass
    return nc


def prep_inputs(times, initial, Wi, bi, Wf0, bf0, Wf1, bf1, Wf2, bf2, Wf3, bf3,
                Wl, bl, t_total=T):
    """Host-side prep. Returns (shared map, per-core state list, accum_ok)."""
    f32 = np.float32
    times = np.asarray(times, f32)
    initial = np.asarray(initial, f32)
    Wi, bi = np.asarray(Wi, f32), np.asarray(bi, f32)
    W0, b0 = np.asarray(Wf0, f32), np.asarray(bf0, f32)
    W1, b1 = np.asarray(Wf1, f32), np.asarray(bf1, f32)
    W2, b2 = np.asarray(Wf2, f32), np.asarray(bf2, f32)
    W3, b3 = np.asarray(Wf3, f32), np.asarray(bf3, f32)
    Wl, bl = np.asarray(Wl, f32), np.asarray(bl, f32)

    GQ = (W3 @ W0).astype(f32)                            # [15, 15]
    GO = (W3 @ Wl).astype(f32)                            # [15, 8]
    cq = (b3 @ W0).astype(f32)                            # [15]
    co = (b3 @ Wl).astype(f32)                            # [8]

    dt = times[1:t_total] - times[:t_total - 1]           # [T-1]
    accum_ok = bool(np.all(dt == dt[0]))

    # initial state: p0 / o0 per batch row
    h0 = initial @ Wi + bi                                # [B, 32]
    p0 = (h0 @ W0).astype(f32)                            # [B, 15]
    o0 = (h0 @ Wl + bl).astype(f32)                       # [B, 8]

    shared = {}

    # ---- accum (stride-16) tensors, dt folded into G
    d0 = float(dt[0]) if accum_ok else 1.0
    w1a = np.zeros((128, 128), f32)
    w2a = np.zeros((128, 128), f32)
    ga = np.zeros((128, 128), f32)
    goa = np.zeros((128, CA), f32)
    bza = np.zeros((128, 4), f32)
    z2a = np.zeros((128, CA), f32)
    z2a[ONES_A, :] = 1.0
    for c in range(NCHA):
        r = 16 * c
        w1a[r:r + HH, r:r + HH] = W1
        w2a[r:r + HH, r:r + HH] = W2
        ga[r:r + HH, r:r + HH] = GQ * d0
        ga[ONES_A, r:r + HH] = cq * d0
        goa[r:r + HH, c * OUT:(c + 1) * OUT] = GO * d0
        goa[ONES_A, c * OUT:(c + 1) * OUT] = co * d0
        bza[r:r + HH, 0] = b0
        bza[r:r + HH, 1] = b1
        bza[r:r + HH, 2] = b2
    shared.update(w1a=w1a, w2a=w2a, ga=ga, goa=goa, bza=bza, z2a=z2a)

    # ---- non-accum (stride-32) tensors, per-step dt applied on DVE
    w1bd = np.zeros((128, 128), f32)
    w2bd = np.zeros((128, 128), f32)
    gbd = np.zeros((128, 128), f32)
    G23 = np.concatenate([GQ, GO], axis=1)                # [15, 23]
    gc23 = np.concatenate([cq, co])                       # [23]
    bzm = np.zeros((128, 4), f32)
    for c in range(NCH):
        r = 32 * c
        w1bd[r:r + HH, r:r + HH] = W1
        w2bd[r:r + HH, r:r + HH] = W2
        gbd[r:r + HH, r:r + HH + 8] = G23
        gbd[ONES_ROW, r:r + HH + 8] = gc23
        bzm[r:r + HH, 0] = b0
        bzm[r:r + HH, 1] = b1
        bzm[r:r + HH, 2] = b2
    z2init = np.zeros((128, 128), f32)
    z2init[ONES_ROW, :] = 1.0
    shared.update(w1bd=w1bd, w2bd=w2bd, gbd=gbd, bz=bzm, z2init=z2init,
                  dts=np.broadcast_to(dt, (128, t_total - 1)).copy())

    core_states = []
    for core in range(NCORES):
        rows0 = core * BSH
        # accum: s0p [128, 64] (p at 16c..16c+14), s0o [128, 64] (o at 8c+j)
        s0p = np.zeros((128, CA), f32)
        s0o = np.zeros((128, CA), f32)
        for c in range(NCHA):
            rows = slice(rows0 + c * CA, rows0 + (c + 1) * CA)
            s0p[16 * c:16 * c + HH, :] = p0[rows].T
            s0o[OUT * c:OUT * (c + 1), :] = o0[rows].T
        # non-accum: s32 [128, 128] (p at 32c.., o at 32c+15..)
        s32 = np.zeros((128, 128), f32)
        for c in range(NCH):
            rows = slice(rows0 + c * 128, rows0 + (c + 1) * 128)
            s32[32 * c:32 * c + HH, :] = p0[rows].T
            s32[32 * c + HH:32 * c + HH + 8, :] = o0[rows].T
        core_states.append({"s0p": s0p, "s0o": s0o, "s32": s32})

    return shared, core_states, accum_ok


def build_cst(shared, cs):
    """Assemble the consolidated per-core constant block [128, CST_COLS]."""
    cst = np.zeros((128, CST_COLS), np.float32)
    cst[:, CO_W1:CO_W1 + 128] = shared["w1a"]
    cst[:, CO_W2:CO_W2 + 128] = shared["w2a"]
    cst[:, CO_G:CO_G + 128] = shared["ga"]
    cst[:, CO_BZ:CO_BZ + 4] = shared["bza"]
    cst[:, CO_ID:CO_ID + 128] = np.eye(128, dtype=np.float32)
    cst[:, CO_S0:CO_S0 + CA] = cs["s0p"]
    cst[:, CO_GO:CO_GO + CA] = shared["goa"]
    cst[:, CO_Z2:CO_Z2 + CA] = shared["z2a"]
    return cst


def unshard(scr_list, accum=True, t_total=T):
    """scratch [(c,o) rows, T*cols] per core -> full output [B, T, OUT]."""
    outs = []
    nch, ncols = (NCHA, CA) if accum else (NCH, 128)
    for scr in scr_list:
        s = scr.reshape(nch, OUT, t_total, ncols)         # [c, o, t, n]
        outs.append(np.ascontiguousarray(s.transpose(0, 3, 2, 1))
                    .reshape(BSH, t_total, OUT))
    return np.concatenate(outs, axis=0)


_CACHE = {}


def _get_program(t_total=T, tbuf=TBUF, nstream=NSTREAM, accum=True,
                 repeats=1):
    key = (t_total, tbuf, nstream, accum, repeats)
    if key not in _CACHE:
        _CACHE[key] = build_program(t_total, tbuf, nstream, accum, repeats)
    return _CACHE[key]


def kernel(**inputs) -> np.ndarray:
    from concourse.bass_utils import run_bass_kernel_spmd

    shared, core_states, accum_ok = prep_inputs(**inputs)
    nc = _get_program(accum=accum_ok)
    in_maps = []
    for core in range(NCORES):
        if accum_ok:
            m = {"cst": build_cst(shared, core_states[core])}
        else:
            m = {k: shared[k] for k in
                 ("w1bd", "w2bd", "gbd", "bz", "z2init", "dts")}
            m["s0"] = core_states[core]["s32"]
        in_maps.append(m)
    res = run_bass_kernel_spmd(nc, in_maps, core_ids=list(range(NCORES)))
    scr_list = [res.results[core]["oscr"] for core in range(NCORES)]
    out = unshard(scr_list, accum=accum_ok)
    if accum_ok:
        # the device emits the o-increment trajectory (po is unseeded; the
        # first mmGo opens the PSUM group); add the constant o0 here
        for core in range(NCORES):
            s0o = core_states[core]["s0o"]          # [8c+j, n]
            o0c = (s0o[:NCHA * OUT].reshape(NCHA, OUT, CA)
                   .transpose(0, 2, 1).reshape(BSH, OUT))
            out[core * BSH:(core + 1) * BSH] += o0c[:, None, :]
    return out
